# revision 16
# baseline (speedup 1.0000x reference)
"""CapsuleLayer (dynamic routing) Trainium2 kernel — v3 (bf16 + custom DVE).

Self-contained: shards the full inputs over 8 NeuronCores (data-parallel over
batch), runs a Bass/Tile kernel per core, gathers the full output.

Shapes (full): u [256, 1152, 8] f32, W [1152, 10, 16, 8] f32 -> v [256, 10, 16].
Per core: B=32 batches, W replicated.

Math (per core, ROUTING_ITERS=3):
  u_hat[b,i,od] = sum_k W[i,od,k] * u[b,i,k]          (od = o*16+d)
  b0 = 0; for t in 0..2: c = softmax(b, o); s = sum_i c*u_hat; v = squash(s);
  if t<2: b += sum_d u_hat*v
The t=0 step has uniform c, so v0 comes from a dense (i,k)-contraction of
ut@wr (no u_hat needed); the routing loop then only runs t=1,2.

Device layouts (i = jj*16+g, jj<72, g<16; partitions in [.]):
  wr  [(g,k)=128, (jj,od)]     bf16 host-pretransposed W
  ut  [(g,k)=128, (jj,b)]      bf16 host-pretransposed u shard
  bdu [(g,k)=128, (j,b8,g')]   bf16 block-diag u, host-packed, DMA-streamed
  u_hat [(b8,g16)=128, (jj,od)] bf16, built by PE: bdu.T @ wr per 8-batch blk
  agreement: fused custom-DVE MAC-cumsum (bf16 in, fp16 out, perf modes
    declared); per-(jj,o) sums recovered as cumsum differences at 16-element
    boundaries (fp32 scan state, so the fp16 rounding stays ~1e-3 relative).
  c build: custom-DVE multiply per b' block row writes block-diag c directly.
  s matmul: lhsT = block-diag c bf16 [(b8,g16),(b8',o)], rhs = u_hat
    -> psum[(b',o), od]
"""

import os
import sys

import numpy as np

for _p in ("/opt/trn_rl_repo", "/root/.axon_site/_ro/trn_rl_repo"):
    if os.path.isdir(_p) and _p not in sys.path:
        sys.path.insert(0, _p)

import ml_dtypes

import concourse.bacc as bacc
import concourse.bass as bass
import concourse.mybir as mybir
import concourse.tile as tile

F32 = mybir.dt.float32
F16 = mybir.dt.float16
BF16 = mybir.dt.bfloat16
NPBF = ml_dtypes.bfloat16

# Problem constants (per core)
B = 32          # local batch (256 / 8 cores)
I = 1152        # in capsules
O = 10          # out capsules
D = 16          # out dim
K = 8           # in dim
JJ = 72         # i groups of 16
G = 16          # group size
OD = O * D      # 160
BB = 8          # batch block (psum/output partition packing)
NBLK = B // BB  # 4
AC = 9          # jj per agreement/scan chunk
NCH = JJ // AC  # 8 chunks


def _register_custom_ops():
    """Register two custom DVE ops, with perf-mode table slots populated so
    the engine may run them in 2x/4x modes on packed 2-byte operands:
      SCAN_MAC4_ANT: out = cumsum(in0*in1) along free dim (fp32 state)
      MUL4X_ANT:     out = in0*in1
    """
    from concourse import dve_ops as dops
    from concourse.dve_spec import AluOp, Spec, Src0, Src1, lower, scan
    from concourse.dve_uop import DveOpSpec

    def reg(name, spec):
        existing = [op for op in dops.OPS if op.name == name]
        if existing:
            return existing[0]
        shas = {}
        specs = {}
        for ver in ("v3", "v4"):
            sp = DveOpSpec(
                name=name,
                opcode=0,
                uops=lower(spec, ver=ver),
                uops_2x=lower(spec, ver=ver),
                uops_2x_2p=lower(spec, ver=ver),
                uops_4x=lower(spec, ver=ver),
                perf_max=3,
                rd1_en=True,
            )
            shas[ver] = sp.sha(ver)
            specs[ver] = sp
        op = dops.DveOp(name, spec, subdim=False, uops_sha=shas)
        dops.OPS.append(op)
        dops.CUSTOM_DVE_SPECS[name] = spec
        dops._SUB_OPCODE_FOR_NAME[name] = dops._CUSTOM_DVE_ROW_BASE + len(dops.OPS) - 1
        assert dops._SUB_OPCODE_FOR_NAME[name] < 0x20
        row = dops.get_dve_sub_opcode(name)
        for ver, sp in specs.items():
            sp.opcode = row
            dops._COMPILE_CACHE[(name, ver)] = sp
        return op

    scan_spec = Spec(
        body=scan(AluOp.ADD, Src0 * Src1),
        reference=lambda in0, in1, c0, c1, c2: np.cumsum(
            np.asarray(in0, np.float32).reshape(in0.shape[0], -1)
            * np.asarray(in1, np.float32).reshape(in1.shape[0], -1),
            axis=-1,
        ).reshape(in0.shape),
    )
    mul_spec = Spec(
        body=Src0 * Src1,
        reference=lambda in0, in1, c0, c1, c2: (
            np.asarray(in0, np.float32) * np.asarray(in1, np.float32)
        ),
    )
    return reg("SCAN_MAC4_ANT", scan_spec), reg("MUL4X_ANT", mul_spec)


_SCAN_MAC4, _MUL4X = _register_custom_ops()


def _custom(nc, op, perf_max=0, **kw):
    # perf-mode slots are declared in the table but the vectorized uop
    # programs are not authored, so keep perf_max=0 (1x) for correctness.
    bi = nc.vector._custom_dve(op, **kw)
    bi.ins.perf_max = perf_max
    return bi


def _ap(base, free_dims, extra_offset=0):
    """AP with the base's partition dim and explicit free [step, count] dims."""
    return bass.AP(
        tensor=base.tensor,
        offset=base.offset + extra_offset,
        ap=[list(base.ap[0])] + [list(d) for d in free_dims],
    )


def _pin_act_table():
    """Make every ACT function we use resolve to the one set containing all
    of them (natural_log_exp_and_others), so bacc hoists a single
    InstLoadActFuncSet instead of thrashing Exp<->Ln sets (~1.3us/load)."""
    from concourse.bacc import get_activation_tables

    tabs = get_activation_tables("gen3")
    keep = "natural_log_exp_and_others"
    if keep not in tabs:
        return
    ours = {
        mybir.ActivationFunctionType.Exp,
        mybir.ActivationFunctionType.Ln,
        mybir.ActivationFunctionType.Square,
        mybir.ActivationFunctionType.Copy,
        mybir.ActivationFunctionType.Identity,
    }
    if not ours <= tabs[keep]:
        return
    for name, s in tabs.items():
        if name != keep:
            s -= ours


def _squash(nc, pool, s_sb, p, v_out):
    """squash over d (16) per o segment. s_sb: [p, 160] f32 sbuf -> v_out."""
    sq = pool.tile([p, OD], F32, tag="sq")
    nc.scalar.square(sq, s_sb)
    nsq = pool.tile([p, O], F32, tag="nsq")
    nc.vector.reduce_sum(
        out=nsq, in_=sq[:].rearrange("p (o d) -> p o d", d=D),
        axis=mybir.AxisListType.X,
    )
    # sqrt(x) = exp(0.5*ln(x)) — keeps ACT on one table set (ln/exp)
    rt = pool.tile([p, O], F32, tag="rt")
    nc.scalar.activation(rt, nsq, mybir.ActivationFunctionType.Ln)
    nc.scalar.activation(rt, rt, mybir.ActivationFunctionType.Exp, scale=0.5)
    nc.vector.tensor_scalar_add(rt, rt, 1e-8)     # + eps
    op1 = pool.tile([p, O], F32, tag="op1")
    nc.vector.tensor_scalar_add(op1, nsq, 1.0)    # 1 + |s|^2
    nc.vector.tensor_mul(op1, op1, rt)            # (1+n)(sqrt+eps)
    rec = pool.tile([p, O], F32, tag="rec")
    nc.vector.reciprocal(rec, op1)
    nc.vector.tensor_mul(rec, rec, nsq)           # n/((1+n)(sqrt+eps))
    nc.vector.tensor_mul(
        v_out[:].rearrange("p (o d) -> p o d", d=D),
        s_sb[:].rearrange("p (o d) -> p o d", d=D),
        _ap(rec[:], [[1, O], [0, D]]),
    )
    return v_out


def build_program():
    _pin_act_table()
    nc = bacc.Bacc("TRN2")
    wr_d = nc.dram_tensor("wr", [128, JJ * OD], BF16, kind="ExternalInput")
    ut_d = nc.dram_tensor("ut", [128, JJ * B], BF16, kind="ExternalInput")
    # block-diag u, host-packed contiguous per (blk, ch): [4, 8, 128, 1152]
    bdu_d = nc.dram_tensor(
        "bdu", [NBLK * NCH * 128, AC * BB * G], BF16, kind="ExternalInput"
    )
    mb_d = nc.dram_tensor("maskb", [128, BB * O], BF16, kind="ExternalInput")
    md_d = nc.dram_tensor("maskd", [128, OD], F32, kind="ExternalInput")
    out_d = nc.dram_tensor("v_out", [B, OD], F32, kind="ExternalOutput")

    with tile.TileContext(nc) as tc:
        with (
            tc.tile_pool(name="persist", bufs=1) as persist,
            tc.tile_pool(name="uhat", bufs=2) as uhat_pool,
            tc.tile_pool(name="bdu", bufs=2) as bdu_pool,
            tc.tile_pool(name="scano", bufs=2) as scano_pool,
            tc.tile_pool(name="blog", bufs=2) as blog_pool,
            tc.tile_pool(name="cbd", bufs=2) as cbd_pool,
            tc.tile_pool(name="soft", bufs=2) as soft_pool,
            tc.tile_pool(name="small", bufs=2) as small,
            tc.tile_pool(name="pb", bufs=4, space="PSUM") as pb_pool,
            tc.tile_pool(name="ps", bufs=2, space="PSUM") as ps_pool,
            tc.tile_pool(name="ps0", bufs=1, space="PSUM") as ps0_pool,
        ):
            # ---- resident loads ----
            wr = persist.tile([128, JJ, OD], BF16)
            for ch in range(NCH):
                nc.sync.dma_start(
                    out=wr[:, ch * AC : (ch + 1) * AC, :],
                    in_=wr_d[:, ch * AC * OD : (ch + 1) * AC * OD].rearrange(
                        "p (a b) -> p a b", b=OD
                    ),
                )
            ut = persist.tile([128, JJ, B], BF16)
            nc.sync.dma_start(
                out=ut, in_=ut_d[:].rearrange("p (a b) -> p a b", b=B)
            )
            maskb = persist.tile([128, BB * O], BF16)
            nc.sync.dma_start(out=maskb, in_=mb_d[:])
            maskd = persist.tile([128, OD], F32)
            nc.sync.dma_start(out=maskd, in_=md_d[:])

            # ---- s0 = 0.1 * sum_i u_hat  (dense (i,k) contraction) ----
            s0_ps = ps0_pool.tile([B, OD], F32)
            for jj in range(JJ):
                nc.tensor.matmul(
                    s0_ps, lhsT=ut[:, jj, :], rhs=wr[:, jj, :],
                    start=(jj == 0), stop=(jj == JJ - 1),
                )
            s0_sb = small.tile([B, OD], F32, tag="s0")
            nc.scalar.activation(
                s0_sb, s0_ps, mybir.ActivationFunctionType.Copy, scale=0.1
            )
            v0 = persist.tile([B, OD], F32, tag="v0")
            _squash(nc, small, s0_sb, B, v0)  # [32, 160]
            v0bf = persist.tile([B, OD], BF16, tag="v0bf")
            nc.scalar.copy(v0bf, v0)
            # t=1 uses v0 for every block: build all vrep tiles up front so
            # they are off the per-block critical path.
            vrep1 = []
            for blk in range(NBLK):
                vb = v0bf[:]
                src = bass.AP(
                    tensor=vb.tensor,
                    offset=vb.offset + blk * BB * vb.ap[0][0],
                    ap=[[vb.ap[0][0], BB], [0, G], [1, OD]],
                )
                vrep0 = persist.tile([128, OD], BF16, tag=f"vr0_{blk}")
                nc.gpsimd.dma_start(out=vrep0, in_=src)
                vr = persist.tile([128, AC * OD], BF16, tag=f"vr1_{blk}")
                nc.vector.tensor_copy(vr, _ap(vrep0[:], [[0, AC], [1, OD]]))
                vrep1.append(vr)

            # ---- per 8-batch block: build u_hat then route ----
            for blk in range(NBLK):
                u_hat = uhat_pool.tile([128, JJ, OD], BF16)
                for ch in range(NCH):  # 9 jj per chunk
                    bdu = bdu_pool.tile([128, AC, BB, G], BF16)
                    nc.sync.dma_start(
                        out=bdu,
                        in_=bdu_d[
                            (blk * NCH + ch) * 128 : (blk * NCH + ch + 1) * 128, :
                        ].rearrange("p (a b g) -> p a b g", b=BB, g=G),
                    )
                    for j3 in range(3):  # 3-jj groups share one psum bank
                        ps = pb_pool.tile([128, 3, OD], F32)
                        for j in range(3):
                            jj = ch * AC + j3 * 3 + j
                            nc.tensor.matmul(
                                ps[:, j, :], lhsT=bdu[:, j3 * 3 + j, :, :],
                                rhs=wr[:, jj, :], start=True, stop=True,
                            )
                        jj0 = ch * AC + j3 * 3
                        nc.scalar.copy(u_hat[:, jj0 : jj0 + 3, :], ps)

                blog = blog_pool.tile([128, JJ, O], F32)
                vcur = None  # [80, 16] f32 tile holding v_t rows for blk
                for t in (1, 2):
                    # -- vrep: v_{t-1} replicated to [(b,g), 9*od] bf16 --
                    if t == 1:
                        vrep = vrep1[blk]
                    else:
                        vcurbf = small.tile([O * BB, D], BF16, tag="vcbf")
                        nc.scalar.copy(vcurbf, vcur)
                        vtmp = small.tile([BB, OD], BF16, tag="vtmp")
                        nc.gpsimd.dma_start(out=vtmp, in_=vcurbf)
                        vt = vtmp[:]
                        src = bass.AP(
                            tensor=vt.tensor, offset=vt.offset,
                            ap=[[vt.ap[0][0], BB], [0, G], [1, OD]],
                        )
                        vrep0 = small.tile([128, OD], BF16, tag="vrep0")
                        nc.gpsimd.dma_start(out=vrep0, in_=src)
                        vrep = small.tile([128, AC * OD], BF16, tag="vrep")
                        nc.vector.tensor_copy(
                            vrep, _ap(vrep0[:], [[0, AC], [1, OD]])
                        )

                    # -- agreement: fused MAC-cumsum per 9-jj chunk --
                    scano = scano_pool.tile([128, NCH, AC * OD], F16)
                    for h in range(NCH):
                        uh2 = u_hat[:, h * AC : (h + 1) * AC, :].rearrange(
                            "p a b -> p (a b)"
                        )
                        _custom(
                            nc, _SCAN_MAC4, perf_max=0,
                            out=scano[:, h, :], in0=uh2, in1=vrep[:],
                        )
                    # batched cumsum-difference extraction into blog (f32):
                    #   hi = scano[.., 16n+15]; blog = hi (t1) / blog+hi (t2)
                    #   blog[.., n>0] -= hi[.., n-1]
                    sv = scano[:]
                    s_hi = _ap(sv, [[AC * OD, NCH], [D, AC * O]], D - 1)
                    s_lo = _ap(sv, [[AC * OD, NCH], [D, AC * O - 1]], D - 1)
                    bl3 = _ap(blog[:], [[AC * O, NCH], [1, AC * O]])
                    bl3s = _ap(blog[:], [[AC * O, NCH], [1, AC * O - 1]], 1)
                    if t == 1:
                        nc.vector.tensor_copy(bl3, s_hi)
                    else:
                        nc.vector.tensor_add(bl3, bl3, s_hi)
                    nc.vector.tensor_sub(bl3s, bl3s, s_lo)

                    # -- c = softmax(blog) over o (unnormalized exp * 1/D) --
                    e_bf = soft_pool.tile([128, JJ, O], BF16, tag="ebf")
                    nc.scalar.activation(
                        e_bf, blog, mybir.ActivationFunctionType.Exp
                    )
                    dsum = small.tile([128, JJ], F32, tag="dsum")
                    nc.vector.reduce_sum(
                        out=dsum, in_=e_bf, axis=mybir.AxisListType.X
                    )
                    drec = small.tile([128, JJ], F32, tag="drec")
                    nc.vector.reciprocal(drec, dsum)
                    e_n = soft_pool.tile([128, JJ, O], BF16, tag="en")
                    nc.gpsimd.tensor_mul(
                        e_n, e_bf, _ap(drec[:], [[1, JJ], [0, O]])
                    )
                    # -- cbd: block-diag c, broadcast multiply split so the
                    #    idle GPSIMD engine carries 2 of the 8 b' rows --
                    cbd = cbd_pool.tile([128, JJ, BB, O], BF16)
                    nc.vector.tensor_mul(
                        cbd[:, :, 0:6, :],
                        _ap(e_n[:], [[O, JJ], [0, 6], [1, O]]),
                        _ap(maskb[:], [[0, JJ], [O, 6], [1, O]]),
                    )
                    nc.gpsimd.tensor_mul(
                        cbd[:, :, 6:8, :],
                        _ap(e_n[:], [[O, JJ], [0, 2], [1, O]]),
                        _ap(maskb[:], [[0, JJ], [O, 2], [1, O]], 6 * O),
                    )

                    # -- s matmul: lhsT = cbd[jj], rhs = u_hat[jj] --
                    s_ps = ps_pool.tile([BB * O, OD], F32)
                    for jj in range(JJ):
                        nc.tensor.matmul(
                            s_ps, lhsT=cbd[:, jj, :, :], rhs=u_hat[:, jj, :],
                            start=(jj == 0), stop=(jj == JJ - 1),
                        )

                    # -- diag extract: s80[(b,o), d] = s_ps[(b,o), o*16+d]
                    #    via constant diag mask + reduce over o' --
                    sdm = small.tile([O * BB, OD], F32, tag="sdm")
                    nc.vector.tensor_mul(sdm, s_ps, maskd[: O * BB, :])
                    s80 = small.tile([O * BB, D], F32, tag="s80")
                    nc.vector.reduce_sum(
                        out=s80,
                        in_=sdm[:].rearrange("p (o d) -> p d o", d=D),
                        axis=mybir.AxisListType.X,
                    )
                    # squash on [(b,o), d] with per-partition scalars
                    nsq = small.tile([O * BB, 1], F32, tag="nsq80")
                    sq = small.tile([O * BB, D], F32, tag="sq80")
                    nc.scalar.square(sq, s80)
                    nc.vector.reduce_sum(
                        out=nsq, in_=sq, axis=mybir.AxisListType.X
                    )
                    # squash factor ~= sqrt(nsq)/(1+nsq)  (eps negligible);
                    # sqrt via exp(0.5*ln) to stay on one ACT table set
                    rt = small.tile([O * BB, 1], F32, tag="rt80")
                    nc.scalar.activation(
                        rt, nsq, mybir.ActivationFunctionType.Ln
                    )
                    nc.scalar.activation(
                        rt, rt, mybir.ActivationFunctionType.Exp, scale=0.5
                    )
                    op1 = small.tile([O * BB, 1], F32, tag="op180")
                    nc.vector.tensor_scalar_add(op1, nsq, 1.0)
                    rec = small.tile([O * BB, 1], F32, tag="rec80")
                    nc.vector.reciprocal(rec, op1)
                    nc.vector.tensor_mul(rec, rec, rt)
                    vcur = small.tile([O * BB, D], F32, tag="vcur")
                    nc.vector.tensor_scalar_mul(vcur, s80, rec)

                # v_out[blk*8+b, o*16+d] = vcur[b*10+o, d] (same flat order)
                nc.gpsimd.dma_start(
                    out=out_d[blk * BB : (blk + 1) * BB, :], in_=vcur
                )
    nc.compile()
    return nc


# ---------------- host side ----------------

_NC_CACHE = None


def _get_nc():
    global _NC_CACHE
    if _NC_CACHE is None:
        _NC_CACHE = build_program()
    return _NC_CACHE


def _pack_wr(W):
    # wr[g*8+k, jj*160 + o*16 + d] = W[jj*16+g, o, d, k]
    return np.ascontiguousarray(
        W.reshape(JJ, G, O, D, K).transpose(1, 4, 0, 2, 3).reshape(128, JJ * OD)
    ).astype(NPBF)


def _pack_ut(u_loc):
    # ut[g*8+k, jj*B + b] = u_loc[b, jj*16+g, k]
    return np.ascontiguousarray(
        u_loc.reshape(B, JJ, G, K).transpose(2, 3, 1, 0).reshape(128, JJ * B)
    ).astype(NPBF)


def _masks():
    p = np.arange(128)
    mb = (np.arange(BB)[None, :] == (p // G)[:, None]).astype(np.float32)
    mb = np.repeat(mb, O, axis=1)  # [128, 80] over (b', o)
    # maskd[(b,o) p<80, o'*16+d] = (o' == o); rows >=80 zero
    md = np.zeros((128, OD), dtype=np.float32)
    po = np.arange(O * BB) % O
    for od in range(OD):
        md[: O * BB, od] = (od // D == po).astype(np.float32)
    return mb.astype(NPBF), md


def _pack_bdu(u_loc):
    # bdu[(blk,ch)*128 + g*8+k, (j, b, g')] = u_loc[blk*8+b, (ch*9+j)*16+g', k]
    #   nonzero only when g' == g; contiguous per (blk, ch) slice.
    u4 = u_loc.reshape(NBLK, BB, NCH, AC, G, K)  # (blk, b, ch, j, g, k)
    out = np.zeros((NBLK, NCH, G, K, AC, BB, G), dtype=NPBF)
    for g in range(G):
        # (blk, ch, k, j, b)
        out[:, :, g, :, :, :, g] = u4[:, :, :, :, g, :].transpose(
            0, 2, 4, 3, 1
        ).astype(NPBF)
    return np.ascontiguousarray(out.reshape(NBLK * NCH * 128, AC * BB * G))


LAST_RESULTS = None


def kernel(u, W):
    from concourse.bass_utils import run_bass_kernel_spmd

    global LAST_RESULTS
    u = np.asarray(u, dtype=np.float32)
    W = np.asarray(W, dtype=np.float32)
    nc = _get_nc()
    wr = _pack_wr(W)
    mb, md = _masks()
    in_maps = []
    for c in range(8):
        u_loc = u[c * B : (c + 1) * B]
        in_maps.append(
            {
                "wr": wr,
                "ut": _pack_ut(u_loc),
                "bdu": _pack_bdu(u_loc),
                "maskb": mb,
                "maskd": md,
            }
        )
    trace = bool(int(os.environ.get("KBENCH_TRACE", "0")))
    try:
        res = run_bass_kernel_spmd(
            nc, in_maps, core_ids=list(range(8)), trace=trace
        )
    except ModuleNotFoundError:
        # axon NTFF hook unavailable in this container; run without trace
        res = run_bass_kernel_spmd(nc, in_maps, core_ids=list(range(8)))
    LAST_RESULTS = res
    outs = [r["v_out"].reshape(B, O, D) for r in res.results]
    return np.concatenate(outs, axis=0).astype(np.float32)
